# revision 43
# baseline (speedup 1.0000x reference)
"""Bass/Trainium2 kernel for nn_Bert_coss (8-core data-parallel over batch).

Computation (per example):
  o1 = relu(X1 @ W.T + b)            [S, H]
  o2 = relu(X2 @ W.T + b)            [S, H]
  o1_doc, o2_doc = mean over S       [H]
  out = sigmoid(relu(concat(o1_doc, o2_doc) @ fd_w.T + fd_b) @ ff_w.T + ff_b)
  scores[s] = o1e[s] . o2_doc   (o1e = o1 ++ o1_doc row), s in 0..S
  att = softmax(scores); output rows 0..S-1 = att[0:S], row S = out.

Key algorithmic simplification: the reference's full [S+1,S+1] co-attention
einsum is only consumed through its last column, so only S+1 dot products
against o2_doc are needed.

Perf structure (v2):
  - X1 path in fp16 (o1 feeds the softmax numerators directly - needs
    precision). X2 path in fp8-e4m3 with DoubleRow double-pumping (0.5
    cyc/row): o2 is only consumed through its doc-mean over S=512, so fp8
    quantization noise averages down by ~sqrt(S) and is harmless.
  - o1 relu evictions on ACT (with accum doc-sums); o2 relu evictions on
    DVE (tensor_scalar add-bias/max-0 with accum_out). Splitting eviction
    across both engines removes the PSUM-recycle stalls that gated the PE
    in the all-ACT version.
  - Head (fd/ff linears) in fp16: fp32 stationary needs two LDWEIGHTS
    per matmul and 4 cyc/row - it was ~3us of pure tail latency.
  - Host pre-arranges X so each SBUF partition's data is one contiguous
    DRAM run (12KB for X1, 3KB for X2) - large DMA packets.
"""

import sys

for _p in ("/opt/trn_rl_repo",):
    if _p not in sys.path:
        sys.path.append(_p)

import numpy as np
import ml_dtypes
from contextlib import ExitStack

import concourse.bass as bass
import concourse.tile as tile
from concourse import bacc, mybir
from concourse import bass_utils

B, S, V, H = 64, 512, 768, 256
NCORES = 8
BL = B // NCORES        # examples per core
KV = V // 128           # contraction chunks for the mlp matmul
MH = H // 128           # output-partition chunks of H
KJ = KV // 2            # fp8 DoubleRow k-pair chunks

F32 = mybir.dt.float32
F16 = mybir.dt.float16
F8 = mybir.dt.float8e4
AF = mybir.ActivationFunctionType
ALU = mybir.AluOpType
DR = mybir.MatmulPerfMode.DoubleRow
NP_F8 = ml_dtypes.float8_e4m3


def _build_kernel(tc):
    nc = tc.nc
    x1t = nc.dram_tensor("x1t", [BL, 128, KV * S], F16, kind="ExternalInput").ap()
    x2t = nc.dram_tensor("x2t", [BL, 128, KV * S], F8, kind="ExternalInput").ap()
    wt = nc.dram_tensor("wt", [V, H], F16, kind="ExternalInput").ap()
    w8 = nc.dram_tensor("w8", [128, KV * H], F8, kind="ExternalInput").ap()
    mlp_b = nc.dram_tensor("mlp_b", [H, 1], F32, kind="ExternalInput").ap()
    fdwt = nc.dram_tensor("fdwt", [2 * H, H], F16, kind="ExternalInput").ap()
    fd_b = nc.dram_tensor("fd_b", [H, 1], F32, kind="ExternalInput").ap()
    ffwt = nc.dram_tensor("ffwt", [H, 1], F16, kind="ExternalInput").ap()
    ff_b = nc.dram_tensor("ff_b", [1, 1], F32, kind="ExternalInput").ap()
    out = nc.dram_tensor("out", [BL, S + 1], F32, kind="ExternalOutput").ap()

    with ExitStack() as ctx:
        const = ctx.enter_context(tc.tile_pool(name="const", bufs=1))

        # weight chunks as separate tiles so the k=0 matmul only depends on
        # the first small DMA; chunks beyond k=1 are issued interleaved with
        # the first X-chunk DMAs (DMA completions are FIFO per queue)
        wt_v = wt.rearrange("(k p) h -> p k h", p=128)
        wt_tiles = []
        for k in range(KV):
            wtk = const.tile([128, H], F16, tag=f"wt{k}")
            wt_tiles.append(wtk)

        def _wt_dma(k):
            # scalar-queue: weight completions don't gate the X stream's FIFO
            nc.scalar.dma_start(wt_tiles[k][:], wt_v[:, k, :])

        for k in range(KV):
            _wt_dma(k)
        w8_sb = const.tile([128, KV * H], F8)
        w8v = w8_sb[:].rearrange("p (j i h) -> p j i h", j=KJ, i=2)
        mlpb_sb = const.tile([128, MH], F32)
        fdwt_sb = const.tile([128, 4 * H], F16)
        fdb_sb = const.tile([128, MH], F32)
        ffwt_sb = const.tile([128, MH], F16)
        ffb_sb = const.tile([1, 1], F32)
        nffb_sb = const.tile([1, 1], F32)
        expwarm = const.tile([1, 1], F32)

        def _mlpb_dma():
            # mlpb before w8: needed at the first eviction (~2.6us before X2)
            nc.scalar.dma_start(
                mlpb_sb[:].rearrange("p (m o) -> p m o", m=MH),
                mlp_b.rearrange("(m p) o -> p m o", p=128),
            )
            nc.scalar.dma_start(w8_sb[:], w8[:, :])
            # dummy Exp so the ACT table set loads during the DMA ramp, but
            # AFTER the wt/w8 DMA issues (the 1.3us table load would delay
            # the weight stream the very first matmuls need)
            nc.scalar.activation(expwarm[:], mlpb_sb[0:1, 0:1], AF.Exp,
                                 scale=0.0)

        def _late_const_dmas():
            # parameters only needed by the end-of-kernel head
            nc.scalar.dma_start(
                fdwt_sb[:].rearrange("p (k h) -> p k h", k=4),
                fdwt.rearrange("(k p) h -> p k h", p=128),
            )
            nc.scalar.dma_start(
                fdb_sb[:].rearrange("p (m o) -> p m o", m=MH),
                fd_b.rearrange("(m p) o -> p m o", p=128),
            )
            nc.scalar.dma_start(
                ffwt_sb[:].rearrange("p (m o) -> p m o", m=MH),
                ffwt.rearrange("(m p) o -> p m o", p=128),
            )
            nc.scalar.dma_start(ffb_sb[:], ff_b[:, :])
            nc.vector.tensor_scalar_mul(nffb_sb[:], ffb_sb[:], -1.0)

        # doc-vector raw sums; column b*4 + kc, kc in (o1m0, o1m1, o2m0, o2m1)
        docs_all = const.tile([128, 4 * BL], F32)
        # zeros operand for the DVE relu (scalar_tensor_tensor max);
        # memset is emitted inside the b==0 block, after the DMA issues
        zeros_sb = const.tile([128, S], F16)

        with ExitStack() as mctx:
            xpool = mctx.enter_context(tc.tile_pool(name="x", bufs=4))
            o1pool = mctx.enter_context(tc.tile_pool(name="o1", bufs=2))
            o2pool = mctx.enter_context(tc.tile_pool(name="o2", bufs=2))
            dpool = mctx.enter_context(tc.tile_pool(name="docs", bufs=2))
            apool = mctx.enter_context(tc.tile_pool(name="att", bufs=3))
            # 6 + 1 + 1 PSUM banks: deep mlp rotation absorbs eviction jitter
            # (ACT/DVE) so the PE never stalls on PSUM WAR and stays at the
            # high-frequency p-state; the scores pools are 1-block pipelined.
            mm_ps = mctx.enter_context(tc.tile_pool(name="mmps", bufs=3, space="PSUM"))
            sc_ps = mctx.enter_context(tc.tile_pool(name="scps", bufs=1, space="PSUM"))
            dd_ps = mctx.enter_context(tc.tile_pool(name="ddps", bufs=1, space="PSUM"))

            def do_scores(b, o1T, dsc, after=None):
                # dd first: its exp can overlap the ssc matvecs
                sdd = dd_ps.tile([1, 1], F32, tag="sdd")
                for hk in range(MH):
                    mm = nc.tensor.matmul(
                        sdd[:],
                        dsc[:, 2 + hk : 3 + hk],
                        dsc[:, hk : hk + 1],
                        start=(hk == 0),
                        stop=(hk == MH - 1),
                    )
                    if after is not None:
                        # keep PE from stalling: order these matvecs after the
                        # next example's dense matmuls (order-only edge)
                        tile.add_dep_helper(
                            mm.ins, after.ins, sync=False,
                            reason="pipeline scores behind next example's mlp",
                        )
                ssc = sc_ps.tile([1, S], F32)
                for hk in range(MH):
                    mm = nc.tensor.matmul(
                        ssc[:],
                        dsc[:, 2 + hk : 3 + hk],
                        o1T[:, hk * S : (hk + 1) * S],
                        start=(hk == 0),
                        stop=(hk == MH - 1),
                    )
                    if after is not None:
                        tile.add_dep_helper(
                            mm.ins, after.ins, sync=False,
                            reason="pipeline scores behind next example's mlp",
                        )
                # softmax on partition 0, straight from PSUM; no max-
                # subtraction (scores are O(25), far inside fp32 exp range)
                att = apool.tile([1, S], F32)
                s1 = apool.tile([1, 1], F32, name="s1")
                nc.scalar.activation(att[:], ssc[:], AF.Exp, accum_out=s1[:])
                edd = apool.tile([1, 1], F32, name="edd")
                nc.scalar.activation(edd[:], sdd[:], AF.Exp)
                stot = apool.tile([1, 1], F32, name="stot")
                nc.vector.tensor_add(stot[:], s1[:], edd[:])
                rs = apool.tile([1, 1], F32, name="rs")
                nc.vector.reciprocal(rs[:], stot[:])
                nc.vector.tensor_scalar_mul(att[:], att[:], rs[:])
                if b == BL - 1:
                    # tail: sync queue is idle, gpsimd would serialize the
                    # final two output DMAs
                    nc.sync.dma_start(out[b : b + 1, 0:S], att[:])
                else:
                    # scalar queue: keeps gpsimd's SWDGE queue input-only so
                    # its (expensive) end-of-kernel drain runs mid-kernel
                    nc.scalar.dma_start(out[b : b + 1, 0:S], att[:])

            NCH = 3               # first example streams in chunks
            KPC = KV // NCH       # k-chunks per dma chunk
            prev = None
            for b in range(BL):
                o1T = o1pool.tile([128, MH * S], F16)
                # ---- X1 (fp16) ----
                x1_sb = xpool.tile([128, KV * S], F16, tag="x1")
                if b == 0:
                    # stream the first example in k-chunks so the first
                    # matmuls start as soon as possible
                    for c in range(NCH):
                        nc.sync.dma_start(
                            x1_sb[:, c * KPC * S : (c + 1) * KPC * S],
                            x1t[b][:, c * KPC * S : (c + 1) * KPC * S],
                        )
                elif b == 1:
                    for c in range(2):
                        nc.sync.dma_start(
                            x1_sb[:, c * 3 * S : (c + 1) * 3 * S],
                            x1t[b][:, c * 3 * S : (c + 1) * 3 * S],
                        )
                else:
                    nc.sync.dma_start(x1_sb[:], x1t[b])
                # ---- X2 (fp8) ----
                x2_sb = xpool.tile([128, KV * S], F8, tag="x2")
                if b == 0:
                    # gpsimd: transfers in parallel with x1(0) on the cold
                    # sync queue; gpsimd's SWDGE queue then stays empty so
                    # its end-of-kernel drain is trivial
                    nc.gpsimd.dma_start(x2_sb[:], x2t[b])
                else:
                    nc.sync.dma_start(x2_sb[:], x2t[b])
                if b == 0:
                    _mlpb_dma()
                    nc.vector.memset(zeros_sb[:], 0.0)
                if b == 1:
                    _late_const_dmas()

                pss = [
                    mm_ps.tile([128, S], F32, tag=f"ps{m}", name=f"ps{m}")
                    for m in range(MH)
                ]
                for k in range(KV):
                    rhs = x1_sb[:, k * S : (k + 1) * S]
                    for m in range(MH):
                        nc.tensor.matmul(
                            pss[m][:],
                            wt_tiles[k][:, m * 128 : (m + 1) * 128],
                            rhs,
                            start=(k == 0),
                            stop=(k == KV - 1),
                        )
                for m in range(MH):
                    nc.scalar.activation(
                        o1T[:, m * S : (m + 1) * S],
                        pss[m][:],
                        AF.Relu,
                        bias=mlpb_sb[:, m : m + 1],
                        accum_out=docs_all[:, b * 4 + m : b * 4 + m + 1],
                    )

                ps2 = [
                    mm_ps.tile([128, S], F32, tag=f"ps{m}", name=f"ps2{m}")
                    for m in range(MH)
                ]
                x2v = x2_sb[:].rearrange("p (j i s) -> p j i s", j=KJ, i=2)
                for j in range(KJ):
                    for m in range(MH):
                        last_mm = nc.tensor.matmul(
                            ps2[m][:],
                            w8v[:, j, :, m * 128 : (m + 1) * 128],
                            x2v[:, j],
                            start=(j == 0),
                            stop=(j == KJ - 1),
                            perf_mode=DR,
                        )
                for m in range(MH):
                    o2scr = o2pool.tile([128, S], F16, tag=f"o2scr{m}",
                                        name="o2scr")
                    kc = 2 + m
                    if b == BL - 1 and m == 1:
                        # tail: run the two o2 evictions on different engines
                        # so the last example's doc vector closes sooner
                        nc.scalar.activation(
                            o2scr[:],
                            ps2[m][:],
                            AF.Relu,
                            bias=mlpb_sb[:, m : m + 1],
                            accum_out=docs_all[:, b * 4 + kc : b * 4 + kc + 1],
                        )
                        continue
                    # relu(x + b) with per-partition sum into the doc column:
                    # out = (in0 add bias) max zeros; accum_out = sum(out)
                    nc.vector.scalar_tensor_tensor(
                        o2scr[:],
                        ps2[m][:],
                        mlpb_sb[:, m : m + 1],
                        zeros_sb[:],
                        op0=ALU.add,
                        op1=ALU.max,
                        accum_out=docs_all[:, b * 4 + kc : b * 4 + kc + 1],
                    )

                if prev is not None:
                    do_scores(*prev, after=last_mm)
                # per-example scaled doc vectors: [o1d0, o1d1, o2d0, o2d1]
                dsc = dpool.tile([128, 4], F16)
                nc.vector.tensor_scalar_mul(
                    dsc[:], docs_all[:, b * 4 : b * 4 + 4], 1.0 / S
                )
                prev = (b, o1T, dsc)

            # ---- tail: last example's scores + the batched head ----
            # docs_sc first on DVE so the head never queues behind the last
            # softmax's DVE ops
            docs_sc = const.tile([128, 4 * BL], F16)
            nc.vector.tensor_scalar_mul(docs_sc[:], docs_all[:], 1.0 / S)
            docs_v = docs_sc[:].rearrange("p (b k) -> p k b", k=4)
            do_scores(*prev)

            h_sb = const.tile([128, MH * BL], F16)
            for m in range(MH):
                ph = mm_ps.tile([128, BL], F32, tag=f"ps{m}", name="ph")
                for kc in range(4):
                    nc.tensor.matmul(
                        ph[:],
                        fdwt_sb[:, kc * H + m * 128 : kc * H + (m + 1) * 128],
                        docs_v[:, kc, :],
                        start=(kc == 0),
                        stop=(kc == 3),
                    )
                # DVE relu: the ACT queue is busy with the last example's
                # softmax exps at this point
                nc.vector.scalar_tensor_tensor(
                    h_sb[:, m * BL : (m + 1) * BL],
                    ph[:],
                    fdb_sb[:, m : m + 1],
                    zeros_sb[:, 0:BL],
                    op0=ALU.add,
                    op1=ALU.max,
                )
            po = dd_ps.tile([1, BL], F32, name="po", tag="sdd")
            for m in range(MH):
                nc.tensor.matmul(
                    po[:],
                    ffwt_sb[:, m : m + 1],
                    h_sb[:, m * BL : (m + 1) * BL],
                    start=(m == 0),
                    stop=(m == MH - 1),
                )
            # sigmoid(x) = 1/(1+exp(-x)) — stays in the Exp table set
            sig_row = const.tile([1, BL], F32)
            nc.scalar.activation(sig_row[:], po[:], AF.Exp,
                                 bias=nffb_sb[0:1, 0:1], scale=-1.0)
            nc.vector.tensor_scalar_add(sig_row[:], sig_row[:], 1.0)
            nc.vector.reciprocal(sig_row[:], sig_row[:])

            # final output column: out[:, S] = sigmoid head values
            # (scalar queue: runs in parallel with the last att DMA on sync)
            nc.scalar.dma_start(
                out[:, S : S + 1],
                sig_row[0:1, :].rearrange("o (b s) -> o b s", b=BL),
            )


_NC_CACHE = None


def _get_nc():
    global _NC_CACHE
    if _NC_CACHE is None:
        nc = bacc.Bacc("TRN2", target_bir_lowering=False, debug=False,
                       num_devices=NCORES)
        with tile.TileContext(nc) as tc:
            _build_kernel(tc)
        nc.compile()
        _NC_CACHE = nc
    return _NC_CACHE


def kernel(output_1, output_2, mlp_w, mlp_b, fd_w, fd_b, ff_w, ff_b):
    output_1 = np.asarray(output_1, dtype=np.float32)
    output_2 = np.asarray(output_2, dtype=np.float32)
    mlp_w = np.asarray(mlp_w, dtype=np.float32)
    mlp_b = np.asarray(mlp_b, dtype=np.float32)
    fd_w = np.asarray(fd_w, dtype=np.float32)
    fd_b = np.asarray(fd_b, dtype=np.float32)
    ff_w = np.asarray(ff_w, dtype=np.float32)
    ff_b = np.asarray(ff_b, dtype=np.float32)

    # shard over batch; X layouts put each partition's data contiguous in DRAM
    # x1t[c,b,p, k*S+s] = X1[c*BL+b, s, k*128+p]   (fp16)
    x1t = np.ascontiguousarray(
        output_1.reshape(NCORES, BL, S, KV, 128).transpose(0, 1, 4, 3, 2)
    ).reshape(NCORES, BL, 128, KV * S).astype(np.float16)
    # x2t[c,b,p, ((j*2+i)*S)+s] = X2[c*BL+b, s, (2j+i)*128+p]   (fp8 e4m3)
    x2t = np.ascontiguousarray(
        output_2.reshape(NCORES, BL, S, KJ, 2, 128).transpose(0, 1, 5, 3, 4, 2)
    ).reshape(NCORES, BL, 128, KV * S).astype(NP_F8)
    wt = np.ascontiguousarray(mlp_w.T).astype(np.float16)  # [V, H]
    # w8[p, (j*2+i)*H + h] = W.T[(2j+i)*128+p, h]   (fp8 e4m3)
    w8 = np.ascontiguousarray(
        mlp_w.T.reshape(KJ, 2, 128, H).transpose(2, 0, 1, 3)
    ).reshape(128, KV * H).astype(NP_F8)
    mlpb = np.ascontiguousarray(mlp_b.reshape(H, 1))
    fdwt = np.ascontiguousarray(fd_w.T).astype(np.float16)  # [2H, H]
    fdb = np.ascontiguousarray(fd_b.reshape(H, 1))
    ffwt = np.ascontiguousarray(ff_w.T).astype(np.float16)  # [H, 1]
    ffb = np.ascontiguousarray(ff_b.reshape(1, 1))

    in_maps = [
        dict(x1t=x1t[c], x2t=x2t[c], wt=wt, w8=w8, mlp_b=mlpb, fdwt=fdwt,
             fd_b=fdb, ffwt=ffwt, ff_b=ffb)
        for c in range(NCORES)
    ]
    global _LAST_IN_MAPS
    _LAST_IN_MAPS = in_maps
    nc = _get_nc()
    res = bass_utils.run_bass_kernel_spmd(nc, in_maps, core_ids=list(range(NCORES)))
    att = np.concatenate([res.results[c]["out"] for c in range(NCORES)], axis=0)
    return np.ascontiguousarray(att.T)  # [S+1, B]
